# revision 10
# baseline (speedup 1.0000x reference)
"""Distributed 2-layer GCN (PyG GCNConv-style) on 8 Trainium2 NeuronCores.

Strategy (v3 — HBM-byte-minimized, per the sharding hint):
  - Nodes are assigned to 160 destination blocks (8 cores x 20 blocks of
    128) by LPT bin-packing on in-degree, so every block has nearly the
    same edge count and KT (edge tiles per block) is minimal.
  - Edges are partitioned by destination block; per (core, dst-block) the
    incoming edges (+ self loops) are packed into KT tiles of 128 edge
    slots.  Host-precomputed sym-normalization coefficients live in
    per-block selection matrices S [128 edge-slots, dst] so scatter-add
    becomes a TensorE matmul.  S is fp8(e4m3) and SBUF-RESIDENT: loaded
    once per program and reused by BOTH layers (no per-pass S traffic).
  - Layer 1 aggregates in INPUT space (A @ x, width F=512) before the W1
    matmul; gathered x rows are fp8 and the aggregation matmuls run in
    DoubleRow mode (pairs of edge tiles; odd KT tail is a single fp8
    matmul).
  - Dense layer 1: the A@x @ W1 path optionally runs as split-fp8
    (ax = ax_hi+ax_lo e4m3, W1 = W1_hi+W1_lo e4m3, 3 cross terms of
    DoubleRow matmuls = 0.75x the bf16 cycles at ~bf16 accuracy); the
    x @ RW1 residual path stays bf16 (fp8 there costs real accuracy).
  - hw = h@W2 and d2 = h@RW2 share one matmul (concatenated rhs); b2
    is folded in on the DVE copy.  hw rows are stored fp8 at a 256B row
    stride (payload duplicated into both halves so the AllGather output
    stays finite); the phase-B gather pulls only the 128B payload
    (elem_step=256, elem_size=128).
  - The halo exchange (AllGather of the hw table) is CHUNKED (4 chunks,
    launched as soon as each chunk's blocks are written) and its output
    tensor is addr_space="Shared" so peers write it directly.
  - Layer 2 aggregates AFTER the W2 projection (width 128) with the
    resident S (DoubleRow fp8).
  - Weights replicated; everything laid out feature-major on the host so
    the kernel needs zero on-device transposes.  Output is bf16.

kernel(**inputs) takes the FULL unsharded inputs and returns the FULL
[20000, 128] float32 output.
"""

import math

import numpy as np
import ml_dtypes

import concourse.bass as bass
import concourse.mybir as mybir
import concourse.tile as tile
from concourse import bacc
from concourse.bass_utils import run_bass_kernel_spmd

# ----------------------------------------------------------------------------
# configuration
# ----------------------------------------------------------------------------
C = 8            # cores
CHUNKS = 4       # AllGather chunks (falls back to 1 if B % CHUNKS)
GB = 2           # dst-blocks per gather call / xg tile
D1 = "mixW"      # dense-1 mode: "bf16" | "mixW" (W1 path split-fp8)
HW_PAIR = True   # hw rows: False = fp8 payload duplicated ("dup"),
                 # True = fp8 hi+lo split pair (better accuracy; gather
                 # payloads must be a multiple of 256B either way)

_DT = {
    "bf16": (mybir.dt.bfloat16, ml_dtypes.bfloat16),
    "f32": (mybir.dt.float32, np.float32),
    "fp8": (mybir.dt.float8e4, ml_dtypes.float8_e4m3),
}
COMPUTE = "bf16"
F8 = ml_dtypes.float8_e4m3

_prog_cache: dict = {}


def _cfg_from_shapes(x, w1, w2):
    n, f = x.shape
    h = w1.shape[1]
    out = w2.shape[1]
    assert n % C == 0, n
    nl = n // C                      # nominal nodes per core
    nlp = ((nl + 127) // 128) * 128  # padded nodes per core
    b = nlp // 128                   # dst blocks per core
    assert f % 128 == 0 and h % 128 == 0 and out % 128 == 0
    return dict(N=n, F=f, H=h, OUT=out, NL=nl, NLP=nlp, B=b, NP=C * nlp,
                FK=f // 128, HC=h // 128, OC=out // 128)


def _split8(a):
    hi = a.astype(F8).astype(np.float32)
    lo = (a - hi).astype(F8)
    return hi.astype(F8), lo


# ----------------------------------------------------------------------------
# host-side preprocessing: graph partition + norm coefficients + layouts
# ----------------------------------------------------------------------------
def _preprocess(x, edge_index, edge_weight, w1, b1, w2, b2, rw1, rb1, rw2, rb2,
                cfg, np_cdt):
    N, F, H, OUT = cfg["N"], cfg["F"], cfg["H"], cfg["OUT"]
    NLP, B, NP = cfg["NLP"], cfg["B"], cfg["NP"]
    HC, FK = cfg["HC"], cfg["FK"]

    row = np.asarray(edge_index[0], dtype=np.int64)
    col = np.asarray(edge_index[1], dtype=np.int64)
    ew = np.asarray(edge_weight, dtype=np.float32)
    x = np.asarray(x, dtype=np.float32)

    # symmetric normalization, exactly like the reference (self loop weight 1)
    deg = np.bincount(col, weights=ew.astype(np.float64), minlength=N) + 1.0
    deg = deg.astype(np.float32)
    dis = np.where(deg > 0, 1.0 / np.sqrt(np.where(deg > 0, deg, 1.0)), 0.0)
    dis = dis.astype(np.float32)

    # ---- LPT balance: assign nodes to C*B dst blocks by slot weight ----
    import heapq
    nbins = C * B
    wt = np.bincount(col, minlength=N) + 1          # in-edges + self loop
    order = np.argsort(-wt, kind="stable")
    heap = [(0, b) for b in range(nbins)]
    heapq.heapify(heap)
    bin_cnt = np.zeros(nbins, dtype=np.int64)
    node_bin = np.zeros(N, dtype=np.int64)
    node_dloc = np.zeros(N, dtype=np.int64)
    for v in order:
        while True:
            load, bb = heapq.heappop(heap)
            if bin_cnt[bb] < 128:
                break
        node_bin[v] = bb
        node_dloc[v] = bin_cnt[bb]
        bin_cnt[bb] += 1
        heapq.heappush(heap, (load + int(wt[v]), bb))
    # bins -> (core, blk): deal bins sorted by load round-robin over cores
    bin_load = np.zeros(nbins, dtype=np.int64)
    np.add.at(bin_load, node_bin, wt)
    bin_order = np.argsort(-bin_load, kind="stable")
    bin_core = np.zeros(nbins, dtype=np.int64)
    bin_blk = np.zeros(nbins, dtype=np.int64)
    core_fill = np.zeros(C, dtype=np.int64)
    for i, bb in enumerate(bin_order):
        c = i % C
        bin_core[bb] = c
        bin_blk[bb] = core_fill[c]
        core_fill[c] += 1
    node_core = bin_core[node_bin]
    node_blk = bin_blk[node_bin]

    loop = np.arange(N, dtype=np.int64)
    srcs = np.concatenate([row, loop])
    dsts = np.concatenate([col, loop])
    norms = np.concatenate([dis[row] * ew * dis[col], dis * dis])

    core = node_core[dsts]
    blk = node_blk[dsts]
    dloc = node_dloc[dsts]

    key = (core * B + blk).astype(np.int64)
    order_e = np.argsort(key, kind="stable")
    key_s = key[order_e]
    counts = np.bincount(key_s, minlength=C * B)
    starts = np.zeros(C * B, dtype=np.int64)
    np.cumsum(counts[:-1], out=starts[1:])
    pos = np.arange(key_s.size, dtype=np.int64) - starts[key_s]

    KT = max(2, int(math.ceil(counts.max() / 128)))

    src_s = srcs[order_e].astype(np.int32)
    core_s = core[order_e]
    blk_s = blk[order_e]
    dloc_s = dloc[order_e]
    norm_s = norms[order_e]
    kt_s = pos // 128
    p_s = pos % 128
    slot = kt_s * 128 + p_s

    # int16 gather indices (phase A): x_table row = original node id
    assert N < 2 ** 15
    idx16 = np.zeros((C, 16, B * KT * 8), dtype=np.int16)
    idx16[core_s, slot % 16, blk_s * (KT * 8) + slot // 16] = \
        src_s.astype(np.int16)
    idx16_all = np.tile(idx16, (1, 8, 1))

    # phase-B gather indices: hw table is CHUNK-major
    nchunk = CHUNKS if B % CHUNKS == 0 else 1
    BC = B // nchunk
    sc = node_core[srcs]
    sb = node_blk[srcs]
    sd = node_dloc[srcs]
    row_b = ((sb // BC) * C * BC * 128 + sc * BC * 128 + (sb % BC) * 128 + sd)
    assert row_b.max() < 2 ** 15
    row_b_s = row_b[order_e].astype(np.int16)
    idx16b = np.zeros((C, 16, B * KT * 8), dtype=np.int16)
    idx16b[core_s, slot % 16, blk_s * (KT * 8) + slot // 16] = row_b_s
    idx16b_all = np.tile(idx16b, (1, 8, 1))

    # S coefficients, partition(slot)-major: S[c, p, b, kt, d], fp8 resident
    S_all = np.zeros((C, 128, B, KT, 128), dtype=np.float32)
    S_all[core_s, p_s, blk_s, kt_s, dloc_s] = norm_s
    s_all = S_all.astype(F8)

    # replicated x gather table [N, F] fp8 (512B rows; stride % 256 == 0)
    x_table = x.astype(F8)

    # per-core padded x in node-slot order [NLP, F]
    x_loc = np.zeros((C, NLP, F), dtype=np.float32)
    x_loc[node_core, node_blk * 128 + node_dloc] = x

    # feature-major, block-major residual x: bf16 [128, B, FK, 128]
    # xt[p, b, k, j] = x_loc[b*128+j, k*128+p]
    xt4 = x_loc.reshape(C, B, 128, FK, 128).transpose(0, 4, 1, 3, 2)
    xt_all = np.ascontiguousarray(xt4).astype(np_cdt)

    w1 = np.asarray(w1, np.float32)
    rw1 = np.asarray(rw1, np.float32)
    w2 = np.asarray(w2, np.float32)
    rw2 = np.asarray(rw2, np.float32)
    b1c = (np.asarray(b1, np.float32) + np.asarray(rb1, np.float32))
    b2c = (np.asarray(b2, np.float32) + np.asarray(rb2, np.float32))

    # [128, FK, H] : w_in[p, k, j] = w[k*128+p, j]
    def fmaj(w):
        return np.ascontiguousarray(w.reshape(FK, 128, H).transpose(1, 0, 2))

    if D1 == "mixW":
        w1h, w1l = _split8(fmaj(w1))
        w1_in = np.stack([w1h, w1l], axis=1)        # [128, 2, FK, H] fp8
    else:
        w1_in = fmaj(w1).astype(np_cdt)
    rw1_in = fmaj(rw1).astype(np_cdt)

    # [128, HC, 2*OUT]: [w2 | rw2]
    w2c = np.concatenate(
        [w2.reshape(HC, 128, OUT), rw2.reshape(HC, 128, OUT)], axis=2)
    w2rw2_in = np.ascontiguousarray(w2c.transpose(1, 0, 2)).astype(np_cdt)

    bias1_in = np.ascontiguousarray(b1c.reshape(HC, 128).T).astype(np.float32)
    b2crep_in = np.ascontiguousarray(
        np.broadcast_to(b2c, (128, OUT))).astype(np.float32)

    in_maps = []
    for c in range(C):
        in_maps.append({
            "x_table": x_table,
            "idx16_in": np.ascontiguousarray(idx16_all[c]),
            "idx16b_in": np.ascontiguousarray(idx16b_all[c]),
            "s_in": np.ascontiguousarray(s_all[c].reshape(128, B, KT * 128)),
            "xt_in": np.ascontiguousarray(
                xt_all[c].reshape(128, B, FK * 128)),
            "w1_in": w1_in,
            "rw1_in": rw1_in,
            "w2rw2_in": w2rw2_in,
            "bias1_in": bias1_in,
            "b2crep_in": b2crep_in,
        })
    # host info to unshard the output
    shard = dict(node_core=node_core, node_blk=node_blk, node_dloc=node_dloc)
    return in_maps, KT, shard


# ----------------------------------------------------------------------------
# device program
# ----------------------------------------------------------------------------
def _build(cfg, KT, cdt, debug_out=False, reps=1, no_collective=False):
    F, H, OUT = cfg["F"], cfg["H"], cfg["OUT"]
    NLP, B, NP = cfg["NLP"], cfg["B"], cfg["NP"]
    FK, HC, OC = cfg["FK"], cfg["HC"], cfg["OC"]
    f32 = mybir.dt.float32
    fp8 = mybir.dt.float8e4
    nchunk = CHUNKS if B % CHUNKS == 0 else 1
    BC = B // nchunk          # blocks per collective chunk
    HWROW = 2 * OUT           # hw row bytes (fp8): payload + dup / lo half
    assert GB * KT * 128 <= 4096

    nc = bacc.Bacc("TRN2", target_bir_lowering=False, debug=False,
                   enable_asserts=False, num_devices=C,
                   dynamic_dma_scratch_size=65536, num_swdge_queues=2)

    x_table = nc.dram_tensor("x_table", [cfg["N"], F], fp8,
                             kind="ExternalInput")
    idx16_in = nc.dram_tensor("idx16_in", [128, B * KT * 8], mybir.dt.int16,
                              kind="ExternalInput")
    idx16b_in = nc.dram_tensor("idx16b_in", [128, B * KT * 8], mybir.dt.int16,
                               kind="ExternalInput")
    s_in = nc.dram_tensor("s_in", [128, B, KT * 128], fp8,
                          kind="ExternalInput")
    xt_in = nc.dram_tensor("xt_in", [128, B, FK * 128], cdt,
                           kind="ExternalInput")
    if D1 == "mixW":
        w1_in = nc.dram_tensor("w1_in", [128, 2, FK, H], fp8,
                               kind="ExternalInput")
    else:
        w1_in = nc.dram_tensor("w1_in", [128, FK, H], cdt,
                               kind="ExternalInput")
    rw1_in = nc.dram_tensor("rw1_in", [128, FK, H], cdt, kind="ExternalInput")
    w2rw2_in = nc.dram_tensor("w2rw2_in", [128, HC, 2 * OUT], cdt,
                              kind="ExternalInput")
    bias1_in = nc.dram_tensor("bias1_in", [128, HC], f32,
                              kind="ExternalInput")
    b2crep_in = nc.dram_tensor("b2crep_in", [128, OUT], f32,
                               kind="ExternalInput")
    out_d = nc.dram_tensor("out", [NLP, OUT], mybir.dt.bfloat16,
                           kind="ExternalOutput")

    def _gather(out_tile, table, idx16_sb, b0, nblk, elem, queue=None,
                elem_step=None, ops=2):
        # `ops` desc-gen calls per block: small calls pipeline deeper in the
        # 4096-descriptor SWDGE ring (a 2816-desc call can't double-buffer)
        for blk in range(nblk):
            b = b0 + blk
            step = KT // ops
            for o in range(ops):
                lo, hi = o * step, (o + 1) * step if o < ops - 1 else KT
                nc.gpsimd.dma_gather(
                    out_ap=out_tile[:, blk * KT + lo:blk * KT + hi],
                    in_ap=table[:],
                    idxs_ap=idx16_sb[:, b * KT * 8 + lo * 8:
                                     b * KT * 8 + hi * 8],
                    num_idxs=(hi - lo) * 128, num_idxs_reg=(hi - lo) * 128,
                    elem_size=elem, elem_step=elem_step, single_packet=False,
                    queue_num=b % 2 if queue is None else queue)

    PAIRS, TAIL = KT // 2, KT % 2

    with tile.TileContext(nc) as tc:
        with (
            tc.tile_pool(name="dram", bufs=1, space="DRAM") as dram,
            tc.tile_pool(name="const", bufs=1) as const,
        ):
            # resident constants
            if D1 == "mixW":
                w1_sb = const.tile([128, 2, FK, H], fp8)
            else:
                w1_sb = const.tile([128, FK, H], cdt)
            nc.sync.dma_start(out=w1_sb[:], in_=w1_in[:])
            rw1_sb = const.tile([128, FK, H], cdt)
            nc.sync.dma_start(out=rw1_sb[:], in_=rw1_in[:])
            w2rw2_sb = const.tile([128, HC, 2 * OUT], cdt)
            nc.sync.dma_start(out=w2rw2_sb[:], in_=w2rw2_in[:])
            bias1_sb = const.tile([128, HC], f32)
            nc.sync.dma_start(out=bias1_sb[:], in_=bias1_in[:])
            b2crep_sb = const.tile([128, OUT], f32)
            nc.sync.dma_start(out=b2crep_sb[:], in_=b2crep_in[:])
            idx16_sb = const.tile([128, B * KT * 8], mybir.dt.int16)
            nc.sync.dma_start(out=idx16_sb[:], in_=idx16_in[:])
            idx16b_sb = const.tile([128, B * KT * 8], mybir.dt.int16)
            nc.sync.dma_start(out=idx16b_sb[:], in_=idx16b_in[:])
            s_sb = const.tile([128, B, KT, 128], fp8)
            nc.sync.dma_start(out=s_sb[:], in_=s_in[:])
            # dense-2 (h@rw2 + b2) results, computed in phase A
            outd_sb = const.tile([128, B, OUT], f32)

            for rep in range(reps):
                hw_locs = [
                    dram.tile([BC * 128, HWROW], fp8, tag=f"hw_loc{g}",
                              name=f"hw_loc{rep}_{g}")
                    for g in range(nchunk)]
                hw_full = dram.tile([C * B * 128, HWROW], fp8,
                                    tag="hw_full", name=f"hw_full{rep}")
                # ---------------- phase A: layer 1 + hw + dense2 ------------
                with (
                    tc.tile_pool(name=f"xg_pool{rep}", bufs=3) as xg_pool,
                    tc.tile_pool(name=f"xt_pool{rep}", bufs=3) as xt_pool,
                    tc.tile_pool(name=f"ax_pool{rep}", bufs=2) as ax_pool,
                    tc.tile_pool(name=f"hstage_pool{rep}", bufs=3) as hstage_pool,
                    tc.tile_pool(name=f"hwsb_pool{rep}", bufs=3) as hwsb_pool,
                    tc.tile_pool(name=f"ax_psum{rep}", bufs=2,
                                 space="PSUM") as ax_psum,
                    tc.tile_pool(name=f"h_psum{rep}", bufs=2,
                                 space="PSUM") as h_psum,
                    tc.tile_pool(name=f"hd_psum{rep}", bufs=2,
                                 space="PSUM") as hd_psum,
                ):
                    for b in range(B):
                        if b % GB == 0:
                            xg = xg_pool.tile([128, GB * KT, F], fp8,
                                              tag="xg")
                            _gather(xg, x_table, idx16_sb, b,
                                    min(GB, B - b), F, ops=2)
                        goff = (b % GB) * KT
                        xt_tile = xt_pool.tile([128, FK, 128], cdt,
                                               tag="xt_tile")
                        nc.sync.dma_start(out=xt_tile[:], in_=xt_in[:, b, :])

                        # aggregation in input space: axT[fc] = Xg_chunk.T @ S
                        psum_ax = ax_psum.tile([128, FK, 128], f32,
                                               tag="psum_ax")
                        for fc in range(FK):
                            fs = slice(fc * 128, (fc + 1) * 128)
                            for t in range(PAIRS):
                                nc.tensor.matmul(
                                    out=psum_ax[:, fc, :],
                                    lhsT=xg[:, goff + 2 * t:goff + 2 * t + 2,
                                            fs],
                                    rhs=s_sb[:, b, 2 * t:2 * t + 2, :],
                                    start=(t == 0),
                                    stop=(TAIL == 0 and t == PAIRS - 1),
                                    perf_mode=mybir.MatmulPerfMode.DoubleRow,
                                )
                            if TAIL:
                                nc.tensor.matmul(
                                    out=psum_ax[:, fc, :],
                                    lhsT=xg[:, goff + KT - 1, fs],
                                    rhs=s_sb[:, b, KT - 1, :],
                                    start=False, stop=True)

                        if D1 == "mixW":
                            axh = ax_pool.tile([128, FK, 128], fp8, tag="axh")
                            axl = ax_pool.tile([128, FK, 128], fp8, tag="axl")
                            nc.vector.tensor_copy(out=axh[:], in_=psum_ax[:])
                            nc.vector.tensor_tensor(
                                out=axl[:], in0=psum_ax[:], in1=axh[:],
                                op=mybir.AluOpType.subtract)
                        else:
                            axT_sb = ax_pool.tile([128, FK, 128], cdt,
                                                  tag="axT_sb")
                            nc.vector.tensor_copy(out=axT_sb[:],
                                                  in_=psum_ax[:])

                        # dense: hT = relu(W1.T @ axT + RW1.T @ xT + b1c)
                        hT_stage = hstage_pool.tile([128, HC, 128], cdt,
                                                    tag="hT_sb")
                        for half in range(2):
                            psum_h = h_psum.tile([128, HC // 2, 128], f32,
                                                 tag="psum_h")
                            for j in range(HC // 2):
                                hc = half * (HC // 2) + j
                                hs = slice(hc * 128, (hc + 1) * 128)
                                if D1 == "mixW":
                                    for hi, op in enumerate(
                                            ((0, axh), (0, axl), (1, axh))):
                                        wi, ax_t = op
                                        for t in range(FK // 2):
                                            nc.tensor.matmul(
                                                out=psum_h[:, j, :],
                                                lhsT=w1_sb[:, wi,
                                                           2 * t:2 * t + 2,
                                                           hs],
                                                rhs=ax_t[:, 2 * t:2 * t + 2,
                                                         :],
                                                start=(hi == 0 and t == 0),
                                                stop=False,
                                                perf_mode=(
                                                    mybir.MatmulPerfMode
                                                    .DoubleRow),
                                            )
                                else:
                                    for k in range(FK):
                                        nc.tensor.matmul(
                                            out=psum_h[:, j, :],
                                            lhsT=w1_sb[:, k, hs],
                                            rhs=axT_sb[:, k, :],
                                            start=(k == 0), stop=False)
                                for k in range(FK):
                                    nc.tensor.matmul(
                                        out=psum_h[:, j, :],
                                        lhsT=rw1_sb[:, k, hs],
                                        rhs=xt_tile[:, k, :],
                                        start=False, stop=(k == FK - 1))
                            hw0 = half * (HC // 2)
                            for j in range(HC // 2):
                                hc = hw0 + j
                                nc.scalar.activation(
                                    out=hT_stage[:, hc, :],
                                    in_=psum_h[:, j, :],
                                    func=mybir.ActivationFunctionType.Relu,
                                    bias=bias1_sb[:, hc:hc + 1], scale=1.0)

                        # [hw | d2] = h @ [W2 | RW2]   (node-major)
                        psum_hd = hd_psum.tile([128, 2 * OUT], f32,
                                               tag="psum_hd")
                        for hc in range(HC):
                            nc.tensor.matmul(
                                out=psum_hd[:],
                                lhsT=hT_stage[:, hc, :],
                                rhs=w2rw2_sb[:, hc, :],
                                start=(hc == 0), stop=(hc == HC - 1))
                        hw_sb = hwsb_pool.tile([128, HWROW], fp8, tag="hw_sb")
                        if HW_PAIR:
                            nc.vector.tensor_copy(out=hw_sb[:, 0:OUT],
                                                  in_=psum_hd[:, 0:OUT])
                            nc.vector.tensor_tensor(
                                out=hw_sb[:, OUT:2 * OUT],
                                in0=psum_hd[:, 0:OUT], in1=hw_sb[:, 0:OUT],
                                op=mybir.AluOpType.subtract)
                        else:
                            nc.vector.tensor_copy(out=hw_sb[:, 0:OUT],
                                                  in_=psum_hd[:, 0:OUT])
                            nc.vector.tensor_copy(out=hw_sb[:, OUT:2 * OUT],
                                                  in_=psum_hd[:, 0:OUT])
                        lw = slice((b % BC) * 128, (b % BC + 1) * 128)
                        nc.sync.dma_start(out=hw_locs[b // BC][lw, :],
                                          in_=hw_sb[:])
                        # dense2 + b2 on the copy out of PSUM
                        nc.vector.tensor_tensor(
                            out=outd_sb[:, b, :],
                            in0=psum_hd[:, OUT:2 * OUT], in1=b2crep_sb[:],
                            op=mybir.AluOpType.add)

                # chunked all-gather of hw: each chunk launches as soon as
                # its blocks' hw rows are written.
                for g in range(nchunk):
                    orows = slice(g * C * BC * 128, (g + 1) * C * BC * 128)
                    if no_collective:
                        nc.scalar.dma_start(
                            out=hw_full[orows, :][0:BC * 128, :],
                            in_=hw_locs[g][:])
                    else:
                        nc.gpsimd.collective_compute(
                            "AllGather",
                            mybir.AluOpType.bypass,
                            replica_groups=[list(range(C))],
                            ins=[hw_locs[g][:].opt()],
                            outs=[hw_full[orows, :].opt()],
                        )

                # ---------------- phase B: layer 2 ----------------
                with (
                    tc.tile_pool(name=f"hwg_pool{rep}", bufs=3) as hwg_pool,
                    tc.tile_pool(name=f"osb_pool{rep}", bufs=3) as osb_pool,
                    tc.tile_pool(name=f"o_psum{rep}", bufs=3,
                                 space="PSUM") as o_psum,
                ):
                    gelem = 2 * OUT if HW_PAIR else OUT
                    for b in range(B):
                        if b % GB == 0:
                            hwg = hwg_pool.tile([128, GB * KT, gelem], fp8,
                                                tag="hwg")
                            _gather(hwg, hw_full, idx16b_sb, b,
                                    min(GB, B - b), gelem,
                                    queue=(b // GB) % 2,
                                    elem_step=HWROW, ops=2)
                        goff = (b % GB) * KT

                        psum_o = o_psum.tile([128, OUT], f32, tag="psum_o")
                        terms = ((0,),) if not HW_PAIR else ((0,), (OUT,))
                        nterm = len(terms)
                        for ti, (off,) in enumerate(terms):
                            for t in range(PAIRS):
                                nc.tensor.matmul(
                                    out=psum_o[:],
                                    lhsT=s_sb[:, b, 2 * t:2 * t + 2, :],
                                    rhs=hwg[:, goff + 2 * t:goff + 2 * t + 2,
                                            off:off + OUT],
                                    start=(ti == 0 and t == 0),
                                    stop=(TAIL == 0 and ti == nterm - 1
                                          and t == PAIRS - 1),
                                    perf_mode=mybir.MatmulPerfMode.DoubleRow,
                                )
                            if TAIL:
                                nc.tensor.matmul(
                                    out=psum_o[:],
                                    lhsT=s_sb[:, b, KT - 1, :],
                                    rhs=hwg[:, goff + KT - 1, off:off + OUT],
                                    start=False,
                                    stop=(ti == nterm - 1))
                        out_sb = osb_pool.tile([128, OUT], mybir.dt.bfloat16,
                                               tag="out_sb")
                        nc.vector.tensor_tensor(
                            out=out_sb[:], in0=psum_o[:],
                            in1=outd_sb[:, b, :],
                            op=mybir.AluOpType.add)
                        nc.sync.dma_start(
                            out=out_d[b * 128:(b + 1) * 128, :],
                            in_=out_sb[:])

    nc.compile()
    return nc


# ----------------------------------------------------------------------------
# entry points
# ----------------------------------------------------------------------------
def _run(inputs, trace=False, compute=None, trace_kwargs=None):
    compute = compute or COMPUTE
    cdt, np_cdt = _DT[compute]
    x = np.asarray(inputs["x"])
    cfg = _cfg_from_shapes(x, np.asarray(inputs["w1"]),
                           np.asarray(inputs["w2"]))
    in_maps, KT, shard = _preprocess(
        x, inputs["edge_index"], inputs["edge_weight"],
        inputs["w1"], inputs["b1"], inputs["w2"], inputs["b2"],
        inputs["rw1"], inputs["rb1"], inputs["rw2"], inputs["rb2"],
        cfg, np_cdt)

    key = (tuple(sorted(cfg.items())), KT, compute)
    nc = _prog_cache.get(key)
    if nc is None:
        nc = _build(cfg, KT, cdt)
        _prog_cache[key] = nc

    res = run_bass_kernel_spmd(
        nc, in_maps, core_ids=list(range(C)), trace=trace,
        **(trace_kwargs or {}))

    out_all = np.stack([np.asarray(res.results[c]["out"])
                        for c in range(C)], axis=0)  # [C, NLP, OUT]
    rows = (shard["node_blk"] * 128 + shard["node_dloc"])
    out = out_all[shard["node_core"], rows].astype(np.float32)
    return np.ascontiguousarray(out), res, shard


def kernel(**inputs) -> np.ndarray:
    out, _, _ = _run(inputs, trace=False)
    return out


# revision 12
# speedup vs baseline: 1.5959x; 1.5959x over previous
"""Distributed 2-layer GCN (PyG GCNConv-style) on 8 Trainium2 NeuronCores.

Strategy (v3 — HBM-byte-minimized, per the sharding hint):
  - Nodes are assigned to 160 destination blocks (8 cores x 20 blocks of
    128) by LPT bin-packing on in-degree, so every block has nearly the
    same edge count and KT (edge tiles per block) is minimal.
  - Edges are partitioned by destination block; per (core, dst-block) the
    incoming edges (+ self loops) are packed into KT tiles of 128 edge
    slots.  Host-precomputed sym-normalization coefficients live in
    per-block selection matrices S [128 edge-slots, dst] so scatter-add
    becomes a TensorE matmul.  S is fp8(e4m3) and SBUF-RESIDENT: loaded
    once per program and reused by BOTH layers (no per-pass S traffic).
  - Layer 1 aggregates in INPUT space (A @ x, width F=512) before the W1
    matmul; gathered x rows are fp8 and the aggregation matmuls run in
    DoubleRow mode (pairs of edge tiles; odd KT tail is a single fp8
    matmul).
  - Dense layer 1: the A@x @ W1 path optionally runs as split-fp8
    (ax = ax_hi+ax_lo e4m3, W1 = W1_hi+W1_lo e4m3, 3 cross terms of
    DoubleRow matmuls = 0.75x the bf16 cycles at ~bf16 accuracy); the
    x @ RW1 residual path stays bf16 (fp8 there costs real accuracy).
  - hw = h@W2 and d2 = h@RW2 share one matmul (concatenated rhs); b2
    is folded in on the DVE copy.  hw rows are stored fp8 at a 256B row
    stride (payload duplicated into both halves so the AllGather output
    stays finite); the phase-B gather pulls only the 128B payload
    (elem_step=256, elem_size=128).
  - The halo exchange (AllGather of the hw table) is CHUNKED (4 chunks,
    launched as soon as each chunk's blocks are written) and its output
    tensor is addr_space="Shared" so peers write it directly.
  - Layer 2 aggregates AFTER the W2 projection (width 128) with the
    resident S (DoubleRow fp8).
  - Weights replicated; everything laid out feature-major on the host so
    the kernel needs zero on-device transposes.  Output is bf16.

kernel(**inputs) takes the FULL unsharded inputs and returns the FULL
[20000, 128] float32 output.
"""

import math

import numpy as np
import ml_dtypes

import concourse.bass as bass
import concourse.mybir as mybir
import concourse.tile as tile
from concourse import bacc
from concourse.bass_utils import run_bass_kernel_spmd

# ----------------------------------------------------------------------------
# configuration
# ----------------------------------------------------------------------------
C = 8            # cores
CHUNKS = 4       # AllGather chunks (falls back to 1 if B % CHUNKS)
GB = 2           # dst-blocks per gather call / xg tile
D1 = "mixW"      # dense-1 mode: "bf16" | "mixW" (W1 path split-fp8)
EVEN_KT = False  # pad KT to even so every aggregation matmul is DoubleRow
HW_PAIR = True   # hw rows: False = fp8 payload duplicated ("dup"),
                 # True = fp8 hi+lo split pair (better accuracy; gather
                 # payloads must be a multiple of 256B either way)

_DT = {
    "bf16": (mybir.dt.bfloat16, ml_dtypes.bfloat16),
    "f32": (mybir.dt.float32, np.float32),
    "fp8": (mybir.dt.float8e4, ml_dtypes.float8_e4m3),
}
COMPUTE = "bf16"
F8 = ml_dtypes.float8_e4m3

_prog_cache: dict = {}


def _cfg_from_shapes(x, w1, w2):
    n, f = x.shape
    h = w1.shape[1]
    out = w2.shape[1]
    assert n % C == 0, n
    nl = n // C                      # nominal nodes per core
    nlp = ((nl + 127) // 128) * 128  # padded nodes per core
    b = nlp // 128                   # dst blocks per core
    assert f % 128 == 0 and h % 128 == 0 and out % 128 == 0
    return dict(N=n, F=f, H=h, OUT=out, NL=nl, NLP=nlp, B=b, NP=C * nlp,
                FK=f // 128, HC=h // 128, OC=out // 128)


def _split8(a):
    hi = a.astype(F8).astype(np.float32)
    lo = (a - hi).astype(F8)
    return hi.astype(F8), lo


# ----------------------------------------------------------------------------
# host-side preprocessing: graph partition + norm coefficients + layouts
# ----------------------------------------------------------------------------
def _preprocess(x, edge_index, edge_weight, w1, b1, w2, b2, rw1, rb1, rw2, rb2,
                cfg, np_cdt):
    N, F, H, OUT = cfg["N"], cfg["F"], cfg["H"], cfg["OUT"]
    NLP, B, NP = cfg["NLP"], cfg["B"], cfg["NP"]
    HC, FK = cfg["HC"], cfg["FK"]

    row = np.asarray(edge_index[0], dtype=np.int64)
    col = np.asarray(edge_index[1], dtype=np.int64)
    ew = np.asarray(edge_weight, dtype=np.float32)
    x = np.asarray(x, dtype=np.float32)

    # symmetric normalization, exactly like the reference (self loop weight 1)
    deg = np.bincount(col, weights=ew.astype(np.float64), minlength=N) + 1.0
    deg = deg.astype(np.float32)
    dis = np.where(deg > 0, 1.0 / np.sqrt(np.where(deg > 0, deg, 1.0)), 0.0)
    dis = dis.astype(np.float32)

    # ---- LPT balance: assign nodes to C*B dst blocks by slot weight ----
    import heapq
    nbins = C * B
    wt = np.bincount(col, minlength=N) + 1          # in-edges + self loop
    order = np.argsort(-wt, kind="stable")
    heap = [(0, b) for b in range(nbins)]
    heapq.heapify(heap)
    bin_cnt = np.zeros(nbins, dtype=np.int64)
    node_bin = np.zeros(N, dtype=np.int64)
    node_dloc = np.zeros(N, dtype=np.int64)
    for v in order:
        while True:
            load, bb = heapq.heappop(heap)
            if bin_cnt[bb] < 128:
                break
        node_bin[v] = bb
        node_dloc[v] = bin_cnt[bb]
        bin_cnt[bb] += 1
        heapq.heappush(heap, (load + int(wt[v]), bb))
    # bins -> (core, blk): deal bins sorted by load round-robin over cores
    bin_load = np.zeros(nbins, dtype=np.int64)
    np.add.at(bin_load, node_bin, wt)
    bin_order = np.argsort(-bin_load, kind="stable")
    bin_core = np.zeros(nbins, dtype=np.int64)
    bin_blk = np.zeros(nbins, dtype=np.int64)
    core_fill = np.zeros(C, dtype=np.int64)
    for i, bb in enumerate(bin_order):
        c = i % C
        bin_core[bb] = c
        bin_blk[bb] = core_fill[c]
        core_fill[c] += 1
    node_core = bin_core[node_bin]
    node_blk = bin_blk[node_bin]

    loop = np.arange(N, dtype=np.int64)
    srcs = np.concatenate([row, loop])
    dsts = np.concatenate([col, loop])
    norms = np.concatenate([dis[row] * ew * dis[col], dis * dis])

    core = node_core[dsts]
    blk = node_blk[dsts]
    dloc = node_dloc[dsts]

    key = (core * B + blk).astype(np.int64)
    order_e = np.argsort(key, kind="stable")
    key_s = key[order_e]
    counts = np.bincount(key_s, minlength=C * B)
    starts = np.zeros(C * B, dtype=np.int64)
    np.cumsum(counts[:-1], out=starts[1:])
    pos = np.arange(key_s.size, dtype=np.int64) - starts[key_s]

    KT = max(2, int(math.ceil(counts.max() / 128)))
    if EVEN_KT and KT % 2:
        KT += 1

    src_s = srcs[order_e].astype(np.int32)
    core_s = core[order_e]
    blk_s = blk[order_e]
    dloc_s = dloc[order_e]
    norm_s = norms[order_e]
    kt_s = pos // 128
    p_s = pos % 128
    slot = kt_s * 128 + p_s

    # int16 gather indices (phase A): x_table row = original node id
    assert N < 2 ** 15
    idx16 = np.zeros((C, 16, B * KT * 8), dtype=np.int16)
    idx16[core_s, slot % 16, blk_s * (KT * 8) + slot // 16] = \
        src_s.astype(np.int16)
    idx16_all = np.tile(idx16, (1, 8, 1))

    # phase-B gather indices: hw table is CHUNK-major
    nchunk = CHUNKS if B % CHUNKS == 0 else 1
    BC = B // nchunk
    sc = node_core[srcs]
    sb = node_blk[srcs]
    sd = node_dloc[srcs]
    row_b = ((sb // BC) * C * BC * 128 + sc * BC * 128 + (sb % BC) * 128 + sd)
    assert row_b.max() < 2 ** 15
    row_b_s = row_b[order_e].astype(np.int16)
    idx16b = np.zeros((C, 16, B * KT * 8), dtype=np.int16)
    idx16b[core_s, slot % 16, blk_s * (KT * 8) + slot // 16] = row_b_s
    idx16b_all = np.tile(idx16b, (1, 8, 1))

    # S coefficients, partition(slot)-major: S[c, p, b, kt, d], fp8 resident
    S_all = np.zeros((C, 128, B, KT, 128), dtype=np.float32)
    S_all[core_s, p_s, blk_s, kt_s, dloc_s] = norm_s
    s_all = S_all.astype(F8)

    # replicated x gather table [N, F] fp8 (512B rows; stride % 256 == 0)
    x_table = x.astype(F8)

    # per-core padded x in node-slot order [NLP, F]
    x_loc = np.zeros((C, NLP, F), dtype=np.float32)
    x_loc[node_core, node_blk * 128 + node_dloc] = x

    # feature-major, block-major residual x: bf16 [128, B, FK, 128]
    # xt[p, b, k, j] = x_loc[b*128+j, k*128+p]
    xt4 = x_loc.reshape(C, B, 128, FK, 128).transpose(0, 4, 1, 3, 2)
    xt_all = np.ascontiguousarray(xt4).astype(np_cdt)

    w1 = np.asarray(w1, np.float32)
    rw1 = np.asarray(rw1, np.float32)
    w2 = np.asarray(w2, np.float32)
    rw2 = np.asarray(rw2, np.float32)
    b1c = (np.asarray(b1, np.float32) + np.asarray(rb1, np.float32))
    b2c = (np.asarray(b2, np.float32) + np.asarray(rb2, np.float32))

    # [128, FK, H] : w_in[p, k, j] = w[k*128+p, j]
    def fmaj(w):
        return np.ascontiguousarray(w.reshape(FK, 128, H).transpose(1, 0, 2))

    if D1 == "mixW":
        w1h, w1l = _split8(fmaj(w1))
        w1_in = np.stack([w1h, w1l], axis=1)        # [128, 2, FK, H] fp8
    else:
        w1_in = fmaj(w1).astype(np_cdt)
    rw1_in = fmaj(rw1).astype(np_cdt)

    # [128, HC, 2*OUT]: [w2 | rw2]
    w2c = np.concatenate(
        [w2.reshape(HC, 128, OUT), rw2.reshape(HC, 128, OUT)], axis=2)
    w2rw2_in = np.ascontiguousarray(w2c.transpose(1, 0, 2)).astype(np_cdt)

    bias1_in = np.ascontiguousarray(b1c.reshape(HC, 128).T).astype(np.float32)
    b2crep_in = np.ascontiguousarray(
        np.broadcast_to(b2c, (128, OUT))).astype(np.float32)

    in_maps = []
    for c in range(C):
        in_maps.append({
            "x_table": x_table,
            "idx16_in": np.ascontiguousarray(idx16_all[c]),
            "idx16b_in": np.ascontiguousarray(idx16b_all[c]),
            "s_in": np.ascontiguousarray(s_all[c].reshape(128, B, KT * 128)),
            "xt_in": np.ascontiguousarray(
                xt_all[c].reshape(128, B, FK * 128)),
            "w1_in": w1_in,
            "rw1_in": rw1_in,
            "w2rw2_in": w2rw2_in,
            "bias1_in": bias1_in,
            "b2crep_in": b2crep_in,
        })
    # host info to unshard the output
    shard = dict(node_core=node_core, node_blk=node_blk, node_dloc=node_dloc)
    return in_maps, KT, shard


# ----------------------------------------------------------------------------
# device program
# ----------------------------------------------------------------------------
def _build(cfg, KT, cdt, debug_out=False, reps=1, no_collective=False):
    F, H, OUT = cfg["F"], cfg["H"], cfg["OUT"]
    NLP, B, NP = cfg["NLP"], cfg["B"], cfg["NP"]
    FK, HC, OC = cfg["FK"], cfg["HC"], cfg["OC"]
    f32 = mybir.dt.float32
    fp8 = mybir.dt.float8e4
    nchunk = CHUNKS if B % CHUNKS == 0 else 1
    BC = B // nchunk          # blocks per collective chunk
    HWROW = 2 * OUT           # hw row bytes (fp8): payload + dup / lo half
    assert GB * KT * 128 <= 4096

    nc = bacc.Bacc("TRN2", target_bir_lowering=False, debug=False,
                   enable_asserts=False, num_devices=C,
                   dynamic_dma_scratch_size=65536, num_swdge_queues=2)

    x_table = nc.dram_tensor("x_table", [cfg["N"], F], fp8,
                             kind="ExternalInput")
    idx16_in = nc.dram_tensor("idx16_in", [128, B * KT * 8], mybir.dt.int16,
                              kind="ExternalInput")
    idx16b_in = nc.dram_tensor("idx16b_in", [128, B * KT * 8], mybir.dt.int16,
                               kind="ExternalInput")
    s_in = nc.dram_tensor("s_in", [128, B, KT * 128], fp8,
                          kind="ExternalInput")
    xt_in = nc.dram_tensor("xt_in", [128, B, FK * 128], cdt,
                           kind="ExternalInput")
    if D1 == "mixW":
        w1_in = nc.dram_tensor("w1_in", [128, 2, FK, H], fp8,
                               kind="ExternalInput")
    else:
        w1_in = nc.dram_tensor("w1_in", [128, FK, H], cdt,
                               kind="ExternalInput")
    rw1_in = nc.dram_tensor("rw1_in", [128, FK, H], cdt, kind="ExternalInput")
    w2rw2_in = nc.dram_tensor("w2rw2_in", [128, HC, 2 * OUT], cdt,
                              kind="ExternalInput")
    bias1_in = nc.dram_tensor("bias1_in", [128, HC], f32,
                              kind="ExternalInput")
    b2crep_in = nc.dram_tensor("b2crep_in", [128, OUT], f32,
                               kind="ExternalInput")
    out_d = nc.dram_tensor("out", [NLP, OUT], mybir.dt.bfloat16,
                           kind="ExternalOutput")

    def _gather(out_tile, table, idx16_sb, b0, nblk, elem, queue=None,
                elem_step=None, ops=2):
        # `ops` desc-gen calls per block: small calls pipeline deeper in the
        # 4096-descriptor SWDGE ring (a 2816-desc call can't double-buffer)
        for blk in range(nblk):
            b = b0 + blk
            step = KT // ops
            for o in range(ops):
                lo, hi = o * step, (o + 1) * step if o < ops - 1 else KT
                nc.gpsimd.dma_gather(
                    out_ap=out_tile[:, blk * KT + lo:blk * KT + hi],
                    in_ap=table[:],
                    idxs_ap=idx16_sb[:, b * KT * 8 + lo * 8:
                                     b * KT * 8 + hi * 8],
                    num_idxs=(hi - lo) * 128, num_idxs_reg=(hi - lo) * 128,
                    elem_size=elem, elem_step=elem_step, single_packet=False,
                    queue_num=b % 2 if queue is None else queue)

    PAIRS, TAIL = KT // 2, KT % 2

    with tile.TileContext(nc) as tc:
        with (
            tc.tile_pool(name="dram", bufs=1, space="DRAM") as dram,
            tc.tile_pool(name="const", bufs=1) as const,
        ):
            # resident constants
            if D1 == "mixW":
                w1_sb = const.tile([128, 2, FK, H], fp8)
            else:
                w1_sb = const.tile([128, FK, H], cdt)
            nc.sync.dma_start(out=w1_sb[:], in_=w1_in[:])
            rw1_sb = const.tile([128, FK, H], cdt)
            nc.sync.dma_start(out=rw1_sb[:], in_=rw1_in[:])
            w2rw2_sb = const.tile([128, HC, 2 * OUT], cdt)
            nc.sync.dma_start(out=w2rw2_sb[:], in_=w2rw2_in[:])
            bias1_sb = const.tile([128, HC], f32)
            nc.sync.dma_start(out=bias1_sb[:], in_=bias1_in[:])
            b2crep_sb = const.tile([128, OUT], f32)
            nc.sync.dma_start(out=b2crep_sb[:], in_=b2crep_in[:])
            idx16_sb = const.tile([128, B * KT * 8], mybir.dt.int16)
            nc.sync.dma_start(out=idx16_sb[:], in_=idx16_in[:])
            idx16b_sb = const.tile([128, B * KT * 8], mybir.dt.int16)
            nc.sync.dma_start(out=idx16b_sb[:], in_=idx16b_in[:])
            s_sb = const.tile([128, B, KT, 128], fp8)
            nc.sync.dma_start(out=s_sb[:], in_=s_in[:])
            # dense-2 (h@rw2 + b2) results, computed in phase A
            outd_sb = const.tile([128, B, OUT], f32)

            for rep in range(reps):
                hw_locs = [
                    dram.tile([BC * 128, HWROW], fp8, tag=f"hw_loc{g}",
                              name=f"hw_loc{rep}_{g}")
                    for g in range(nchunk)]
                hw_full = dram.tile([C * B * 128, HWROW], fp8,
                                    tag="hw_full", name=f"hw_full{rep}")
                # ---------------- phase A: layer 1 + hw + dense2 ------------
                with (
                    tc.tile_pool(name=f"xg_pool{rep}", bufs=3) as xg_pool,
                    tc.tile_pool(name=f"xt_pool{rep}", bufs=3) as xt_pool,
                    tc.tile_pool(name=f"ax_pool{rep}", bufs=2) as ax_pool,
                    tc.tile_pool(name=f"hstage_pool{rep}", bufs=3) as hstage_pool,
                    tc.tile_pool(name=f"hwsb_pool{rep}", bufs=3) as hwsb_pool,
                    tc.tile_pool(name=f"ax_psum{rep}", bufs=2,
                                 space="PSUM") as ax_psum,
                    tc.tile_pool(name=f"h_psum{rep}", bufs=2,
                                 space="PSUM") as h_psum,
                    tc.tile_pool(name=f"hd_psum{rep}", bufs=2,
                                 space="PSUM") as hd_psum,
                ):
                    for b in range(B):
                        if b % GB == 0:
                            xg = xg_pool.tile([128, GB * KT, F], fp8,
                                              tag="xg")
                            _gather(xg, x_table, idx16_sb, b,
                                    min(GB, B - b), F, ops=2)
                        goff = (b % GB) * KT
                        xt_tile = xt_pool.tile([128, FK, 128], cdt,
                                               tag="xt_tile")
                        nc.sync.dma_start(out=xt_tile[:], in_=xt_in[:, b, :])

                        # aggregation in input space: axT[fc] = Xg_chunk.T @ S
                        psum_ax = ax_psum.tile([128, FK, 128], f32,
                                               tag="psum_ax")
                        for fc in range(FK):
                            fs = slice(fc * 128, (fc + 1) * 128)
                            for t in range(PAIRS):
                                nc.tensor.matmul(
                                    out=psum_ax[:, fc, :],
                                    lhsT=xg[:, goff + 2 * t:goff + 2 * t + 2,
                                            fs],
                                    rhs=s_sb[:, b, 2 * t:2 * t + 2, :],
                                    start=(t == 0),
                                    stop=(TAIL == 0 and t == PAIRS - 1),
                                    perf_mode=mybir.MatmulPerfMode.DoubleRow,
                                )
                            if TAIL:
                                nc.tensor.matmul(
                                    out=psum_ax[:, fc, :],
                                    lhsT=xg[:, goff + KT - 1, fs],
                                    rhs=s_sb[:, b, KT - 1, :],
                                    start=False, stop=True)

                        if D1 == "mixW":
                            axh = ax_pool.tile([128, FK, 128], fp8, tag="axh")
                            axl = ax_pool.tile([128, FK, 128], fp8, tag="axl")
                            nc.vector.tensor_copy(out=axh[:], in_=psum_ax[:])
                            nc.vector.tensor_tensor(
                                out=axl[:], in0=psum_ax[:], in1=axh[:],
                                op=mybir.AluOpType.subtract)
                        else:
                            axT_sb = ax_pool.tile([128, FK, 128], cdt,
                                                  tag="axT_sb")
                            nc.vector.tensor_copy(out=axT_sb[:],
                                                  in_=psum_ax[:])

                        # dense: hT = relu(W1.T @ axT + RW1.T @ xT + b1c)
                        hT_stage = hstage_pool.tile([128, HC, 128], cdt,
                                                    tag="hT_sb")
                        for half in range(2):
                            psum_h = h_psum.tile([128, HC // 2, 128], f32,
                                                 tag="psum_h")
                            for j in range(HC // 2):
                                hc = half * (HC // 2) + j
                                hs = slice(hc * 128, (hc + 1) * 128)
                                if D1 == "mixW":
                                    for hi, op in enumerate(
                                            ((0, axh), (0, axl), (1, axh))):
                                        wi, ax_t = op
                                        for t in range(FK // 2):
                                            nc.tensor.matmul(
                                                out=psum_h[:, j, :],
                                                lhsT=w1_sb[:, wi,
                                                           2 * t:2 * t + 2,
                                                           hs],
                                                rhs=ax_t[:, 2 * t:2 * t + 2,
                                                         :],
                                                start=(hi == 0 and t == 0),
                                                stop=False,
                                                perf_mode=(
                                                    mybir.MatmulPerfMode
                                                    .DoubleRow),
                                            )
                                else:
                                    for k in range(FK):
                                        nc.tensor.matmul(
                                            out=psum_h[:, j, :],
                                            lhsT=w1_sb[:, k, hs],
                                            rhs=axT_sb[:, k, :],
                                            start=(k == 0), stop=False)
                                for k in range(FK):
                                    nc.tensor.matmul(
                                        out=psum_h[:, j, :],
                                        lhsT=rw1_sb[:, k, hs],
                                        rhs=xt_tile[:, k, :],
                                        start=False, stop=(k == FK - 1))
                            hw0 = half * (HC // 2)
                            for j in range(HC // 2):
                                hc = hw0 + j
                                nc.scalar.activation(
                                    out=hT_stage[:, hc, :],
                                    in_=psum_h[:, j, :],
                                    func=mybir.ActivationFunctionType.Relu,
                                    bias=bias1_sb[:, hc:hc + 1], scale=1.0)

                        # [hw | d2] = h @ [W2 | RW2]   (node-major)
                        psum_hd = hd_psum.tile([128, 2 * OUT], f32,
                                               tag="psum_hd")
                        for hc in range(HC):
                            nc.tensor.matmul(
                                out=psum_hd[:],
                                lhsT=hT_stage[:, hc, :],
                                rhs=w2rw2_sb[:, hc, :],
                                start=(hc == 0), stop=(hc == HC - 1))
                        hw_sb = hwsb_pool.tile([128, HWROW], fp8, tag="hw_sb")
                        if HW_PAIR:
                            nc.vector.tensor_copy(out=hw_sb[:, 0:OUT],
                                                  in_=psum_hd[:, 0:OUT])
                            nc.vector.tensor_tensor(
                                out=hw_sb[:, OUT:2 * OUT],
                                in0=psum_hd[:, 0:OUT], in1=hw_sb[:, 0:OUT],
                                op=mybir.AluOpType.subtract)
                        else:
                            nc.vector.tensor_copy(out=hw_sb[:, 0:OUT],
                                                  in_=psum_hd[:, 0:OUT])
                            nc.vector.tensor_copy(out=hw_sb[:, OUT:2 * OUT],
                                                  in_=psum_hd[:, 0:OUT])
                        lw = slice((b % BC) * 128, (b % BC + 1) * 128)
                        nc.sync.dma_start(out=hw_locs[b // BC][lw, :],
                                          in_=hw_sb[:])
                        # dense2 + b2 on the copy out of PSUM
                        nc.vector.tensor_tensor(
                            out=outd_sb[:, b, :],
                            in0=psum_hd[:, OUT:2 * OUT], in1=b2crep_sb[:],
                            op=mybir.AluOpType.add)

                # chunked all-gather of hw: each chunk launches as soon as
                # its blocks' hw rows are written.
                for g in range(nchunk):
                    orows = slice(g * C * BC * 128, (g + 1) * C * BC * 128)
                    if no_collective:
                        nc.scalar.dma_start(
                            out=hw_full[orows, :][0:BC * 128, :],
                            in_=hw_locs[g][:])
                    else:
                        nc.gpsimd.collective_compute(
                            "AllGather",
                            mybir.AluOpType.bypass,
                            replica_groups=[list(range(C))],
                            ins=[hw_locs[g][:].opt()],
                            outs=[hw_full[orows, :].opt()],
                        )

                # ---------------- phase B: layer 2 ----------------
                with (
                    tc.tile_pool(name=f"hwg_pool{rep}", bufs=3) as hwg_pool,
                    tc.tile_pool(name=f"osb_pool{rep}", bufs=3) as osb_pool,
                    tc.tile_pool(name=f"o_psum{rep}", bufs=3,
                                 space="PSUM") as o_psum,
                ):
                    gelem = 2 * OUT if HW_PAIR else OUT
                    for b in range(B):
                        if b % GB == 0:
                            hwg = hwg_pool.tile([128, GB * KT, gelem], fp8,
                                                tag="hwg")
                            _gather(hwg, hw_full, idx16b_sb, b,
                                    min(GB, B - b), gelem,
                                    queue=(b // GB) % 2,
                                    elem_step=HWROW, ops=2)
                        goff = (b % GB) * KT

                        psum_o = o_psum.tile([128, OUT], f32, tag="psum_o")
                        terms = ((0,),) if not HW_PAIR else ((0,), (OUT,))
                        nterm = len(terms)
                        for ti, (off,) in enumerate(terms):
                            for t in range(PAIRS):
                                nc.tensor.matmul(
                                    out=psum_o[:],
                                    lhsT=s_sb[:, b, 2 * t:2 * t + 2, :],
                                    rhs=hwg[:, goff + 2 * t:goff + 2 * t + 2,
                                            off:off + OUT],
                                    start=(ti == 0 and t == 0),
                                    stop=(TAIL == 0 and ti == nterm - 1
                                          and t == PAIRS - 1),
                                    perf_mode=mybir.MatmulPerfMode.DoubleRow,
                                )
                            if TAIL:
                                nc.tensor.matmul(
                                    out=psum_o[:],
                                    lhsT=s_sb[:, b, KT - 1, :],
                                    rhs=hwg[:, goff + KT - 1, off:off + OUT],
                                    start=False,
                                    stop=(ti == nterm - 1))
                        out_sb = osb_pool.tile([128, OUT], mybir.dt.bfloat16,
                                               tag="out_sb")
                        nc.vector.tensor_tensor(
                            out=out_sb[:], in0=psum_o[:],
                            in1=outd_sb[:, b, :],
                            op=mybir.AluOpType.add)
                        nc.sync.dma_start(
                            out=out_d[b * 128:(b + 1) * 128, :],
                            in_=out_sb[:])

    nc.compile()
    return nc


# ----------------------------------------------------------------------------
# entry points
# ----------------------------------------------------------------------------
def _run(inputs, trace=False, compute=None, trace_kwargs=None):
    compute = compute or COMPUTE
    cdt, np_cdt = _DT[compute]
    x = np.asarray(inputs["x"])
    cfg = _cfg_from_shapes(x, np.asarray(inputs["w1"]),
                           np.asarray(inputs["w2"]))
    in_maps, KT, shard = _preprocess(
        x, inputs["edge_index"], inputs["edge_weight"],
        inputs["w1"], inputs["b1"], inputs["w2"], inputs["b2"],
        inputs["rw1"], inputs["rb1"], inputs["rw2"], inputs["rb2"],
        cfg, np_cdt)

    key = (tuple(sorted(cfg.items())), KT, compute)
    nc = _prog_cache.get(key)
    if nc is None:
        nc = _build(cfg, KT, cdt)
        _prog_cache[key] = nc

    res = run_bass_kernel_spmd(
        nc, in_maps, core_ids=list(range(C)), trace=trace,
        **(trace_kwargs or {}))

    out_all = np.stack([np.asarray(res.results[c]["out"])
                        for c in range(C)], axis=0)  # [C, NLP, OUT]
    rows = (shard["node_blk"] * 128 + shard["node_dloc"])
    out = out_all[shard["node_core"], rows].astype(np.float32)
    return np.ascontiguousarray(out), res, shard


def kernel(**inputs) -> np.ndarray:
    out, _, _ = _run(inputs, trace=False)
    return out
